# revision 4
# baseline (speedup 1.0000x reference)
"""MLA-style attention GPT block on 8 Trainium2 NeuronCores.

Sharding: tensor-parallel over heads x data-parallel over batch.
Core c handles batch b = c // 4 and heads [4*hg, 4*hg+4) with hg = c % 4.
Each core computes a partial c_proj output (2048, 1024) over its 4 heads;
the host sums the 4 partials per batch.

Design notes:
- q/k up-projections (and rope's signed permutation) are folded into the
  down-projection on the host: one fused weight [C, 1024] per core in fp8
  (x64 into e4m3 range; descale folded into the PSUM-evacuation copies,
  the q side also carrying 1/sqrt(DH)).  The rot rows are zero-padded
  into partition alignment so the rope add needs no partition shift.
- v is produced DIRECTLY in attention layout ([keys, dh]) by swapping
  matmul operands (x chunk stationary, fused Wv moving); fp8 precision
  is recovered with residual passes: 64*v = (x8+r8)'w8 + x8's8.  The
  x64 descale rides the softmax-denominator ones row.
- Down-proj and v-proj run as fp8 DoubleRow matmuls (K=256/instruction,
  0.5 cycles/row); scores/att@V/c_proj run in bf16 (the value path does
  not tolerate fp8: relative quantization error survives contractions).
- Causal structure: per 512-query chunk, off-diagonal 128-key blocks are
  computed in [128,1024] PSUM pairs (one exp each); the diagonal band is
  split at 128-query granularity into 10 small matmuls in 3 PSUM tiles
  (3 exps), leaving one [128,512] mask multiply per (head, chunk) on DVE.
- Softmax normalization per (head, q-chunk): the denominator row comes
  free from a 64.0-row in the att@V matmul, is reciprocaled on DVE and
  broadcast across partitions by gpsimd.partition_broadcast; odd heads
  stage the normalize product and use a shifted TensorCopy into the
  c_proj layout (TensorTensor ops require same-start partitions).
- q-chunks are processed in interleaved pairs (3,0), (2,1) so thin
  chunks hide under heavy ones; each pair's c_proj is spread through the
  next pair's units, and the tail c_proj rotates through both PSUM rings.
"""

import sys

sys.path.insert(0, "/opt/trn_rl_repo")

import ml_dtypes
import numpy as np

import concourse.bass as bass
import concourse.tile as tile
from concourse import bacc
from concourse import mybir
from concourse.bass_utils import run_bass_kernel_spmd

B, T, C = 2, 2048, 1024
H, L = 16, 64
DH = 64
DHE = 32
THETA = 10000.0

HG = 4    # head-groups (cores per batch)
HPG = H // HG   # heads per core = 4
FT = 2    # head-pair tiles

KC = 4    # down-proj contraction chunks of 256 (DoubleRow)
TC = 4    # chunks of 512 along T
QB = 4    # query chunks of 512
KB = 16   # key blocks of 128
VW = 1040  # vaug stride per head (16 blocks x 65)

WSC = 64.0                  # fp8 range rescale for the fused q/k weights
QDESC = 1.0 / (WSC * 8.0)   # q descale: weight rescale + 1/sqrt(DH)
KDESC = 1.0 / WSC

F32 = mybir.dt.float32
F32R = mybir.dt.float32r
BF16 = mybir.dt.bfloat16
FP8 = mybir.dt.float8e4
DR = mybir.MatmulPerfMode.DoubleRow
EXP = mybir.ActivationFunctionType.Exp

_NC_CACHE = {}


def _build_nc():
    if "nc" in _NC_CACHE:
        return _NC_CACHE["nc"]
    nc = bacc.Bacc("TRN2", target_bir_lowering=False)

    xT8 = nc.dram_tensor("xT8", [C, T], FP8, kind="ExternalInput")
    dw8 = nc.dram_tensor("dw8", [KC, 128, 2 * 1024], FP8, kind="ExternalInput")
    xr8 = nc.dram_tensor("xr8", [C, T], FP8, kind="ExternalInput")
    wv8 = nc.dram_tensor("wv8", [KC, 128, 2 * 256], FP8, kind="ExternalInput")
    ws8 = nc.dram_tensor("ws8", [KC, 128, 2 * 256], FP8, kind="ExternalInput")
    cosb = nc.dram_tensor("cosb", [128, T], BF16, kind="ExternalInput")
    sinb = nc.dram_tensor("sinb", [128, T], BF16, kind="ExternalInput")
    mask = nc.dram_tensor("mask", [128, 512], BF16, kind="ExternalInput")
    wcsb = nc.dram_tensor("wcsb", [128, 2 * C], BF16, kind="ExternalInput")
    out = nc.dram_tensor("out", [T, C], BF16, kind="ExternalOutput")

    with tile.TileContext(nc) as tc:
        _emit(nc, tc, xT8, xr8, dw8, wv8, ws8, cosb, sinb, mask, wcsb, out)
    nc.compile()

    _NC_CACHE["nc"] = nc
    return nc


def _emit(nc, tc, xT8, xr8, dw8, wv8, ws8, cosb, sinb, mask, wcsb, out):
    from contextlib import ExitStack

    ctx = ExitStack()
    with ctx:
        consts = ctx.enter_context(tc.tile_pool(name="consts", bufs=1))
        persist = ctx.enter_context(tc.tile_pool(name="persist", bufs=1))

        # ---- persistent activations ----
        qfin = [persist.tile([128, T], BF16, tag=f"qfin{t}", name=f"qfin{t}") for t in range(FT)]
        kfin = [persist.tile([128, T], BF16, tag=f"kfin{t}", name=f"kfin{t}") for t in range(FT)]
        vaug = persist.tile([128, HPG * VW], BF16, tag="vaug", name="vaug")
        ycoreb = persist.tile([128, 2 * T], BF16, tag="ycoreb", name="ycoreb")
        y3 = ycoreb.rearrange("p (t x) -> p t x", t=2)
        va4 = vaug.rearrange("p (h b c) -> p (h b) c", h=HPG, c=65)
        nc.gpsimd.memset(va4[:, :, DH], 64.0)

        # ================= projection phase =================
        with tc.tile_pool(name="dwp", bufs=1) as dwp, \
             tc.tile_pool(name="qk_ps", bufs=1, space="PSUM") as qkps, \
             tc.tile_pool(name="v_ps", bufs=2, space="PSUM") as vps, \
             tc.tile_pool(name="proj_sb", bufs=2) as psb, \
             tc.tile_pool(name="xpieces", bufs=3) as xpool:
            # critical-path-first DMA order: x chunk 0 + weights, then consts
            def load_x(tsl):
                xps = []
                for kp in range(2):
                    xp = xpool.tile([128, 4 * 512], FP8, tag=f"xp{kp}", name="xp")
                    src = xT8[kp * 512:(kp + 1) * 512, tsl]
                    nc.sync.dma_start(
                        xp.rearrange("p (i t x) -> p i t x", i=2, t=2),
                        src.rearrange("(i t p) x -> p i t x", i=2, t=2))
                    xps.append(xp)
                for kp in range(2):
                    xr = xpool.tile([128, 4 * 512], FP8, tag=f"xr{kp}", name="xr")
                    src = xr8[kp * 512:(kp + 1) * 512, tsl]
                    nc.sync.dma_start(
                        xr.rearrange("p (i t x) -> p i t x", i=2, t=2),
                        src.rearrange("(i t p) x -> p i t x", i=2, t=2))
                    xps.append(xr)
                return xps

            # interleave so matmul (g, kc) can start as soon as its
            # operands land: xp[kp0], dw0, dw1, xp[kp1], dw2, dw3, wv
            xps = []
            dwt = []
            for kp in range(2):
                xp = xpool.tile([128, 4 * 512], FP8, tag=f"xp{kp}", name="xp")
                src = xT8[kp * 512:(kp + 1) * 512, 0:512]
                nc.sync.dma_start(
                    xp.rearrange("p (i t x) -> p i t x", i=2, t=2),
                    src.rearrange("(i t p) x -> p i t x", i=2, t=2))
                xps.append(xp)
                for k in (2 * kp, 2 * kp + 1):
                    dw = dwp.tile([128, 2 * 1024], FP8, tag=f"dw{k}", name=f"dw{k}")
                    nc.sync.dma_start(dw, dw8[k, :, :])
                    dwt.append(dw)
            for kp in range(2):
                xr = xpool.tile([128, 4 * 512], FP8, tag=f"xr{kp}", name="xr")
                srcr = xr8[kp * 512:(kp + 1) * 512, 0:512]
                nc.sync.dma_start(
                    xr.rearrange("p (i t x) -> p i t x", i=2, t=2),
                    srcr.rearrange("(i t p) x -> p i t x", i=2, t=2))
                xps.append(xr)
            wvt = []
            wst = []
            for kp in range(2):
                wv = dwp.tile([128, 2 * 512], FP8, tag=f"wv{kp}", name=f"wv{kp}")
                nc.sync.dma_start(
                    wv.rearrange("p (i r) -> p i r", i=2),
                    wv8[2 * kp:2 * kp + 2, :, :].rearrange("i p r -> p i r"))
                wvt.append(wv)
                ws = dwp.tile([128, 2 * 512], FP8, tag=f"ws{kp}", name=f"ws{kp}")
                nc.sync.dma_start(
                    ws.rearrange("p (i r) -> p i r", i=2),
                    ws8[2 * kp:2 * kp + 2, :, :].rearrange("i p r -> p i r"))
                wst.append(ws)
            # rope tables (shared q/k; descale rides the evacuation copies)
            cos_sb = consts.tile([128, T], BF16, tag="cosb", name="cos_sb")
            sin_sb = consts.tile([128, T], BF16, tag="sinb", name="sin_sb")
            tables = ((cos_sb, cosb), (sin_sb, sinb))

            def load_tables(tsl):
                for dst, src in tables:
                    nc.sync.dma_start(dst[:, tsl], src[:, tsl])

            load_tables(slice(0, 512))
            xps_pre = [load_x(slice(512, 1024)), load_x(slice(1024, 1536))]
            load_tables(slice(512, 1024))
            load_tables(slice(1024, 1536))
            mask_sb = consts.tile([128, 512], BF16, tag="mask", name="mask_sb")
            wcs_sb = consts.tile([128, 2 * C], BF16, tag="wcsb", name="wcs_sb")
            nc.sync.dma_start(mask_sb, mask[:, :])
            nc.sync.dma_start(wcs_sb, wcsb[:, :])

            for t in range(TC):
                tsl = slice(t * 512, (t + 1) * 512)
                if t in (1, 2):
                    xps = xps_pre[t - 1]
                elif t > 0:
                    xps = load_x(tsl)
                    load_tables(tsl)
                xv = [xp.rearrange("p (i t x) -> p i t x", i=2, t=2) for xp in xps[:2]]

                # 4 accumulating groups per side: T1 T2 T3a T3b; k side
                # reuses the q banks through the bufs=1 ring.
                def side(base, tags):
                    grp = [qkps.tile([128, 512], F32, tag=f"g{i}", name=f"g{i}")
                           for i in tags]
                    for g in range(4):
                        for kc in range(KC):
                            kp, i = kc // 2, kc % 2
                            dw3 = dwt[kc].rearrange("p (t n) -> p t n", t=2)
                            nc.tensor.matmul(
                                grp[g],
                                lhsT=dw3[:, :, (base + g) * 128:(base + g + 1) * 128],
                                rhs=xv[kp][:, i, :, :],
                                start=(kc == 0), stop=(kc == KC - 1),
                                perf_mode=DR)
                    return grp

                def vproj(tb):
                    # 64*v = (x8 + r8)' w8 + x8' s8  (fp8 DoubleRow passes)
                    blk = 4 * t + tb
                    vp = vps.tile([128, 256], F32, tag="vp", name="vp")
                    xrv = [xps[2].rearrange("p (i t x) -> p i t x", i=2, t=2),
                           xps[3].rearrange("p (i t x) -> p i t x", i=2, t=2)]
                    for pi, (xop, wop) in enumerate(
                            ((xv, wvt), (xrv, wvt), (xv, wst))):
                        for kc in range(KC):
                            kp, i = kc // 2, kc % 2
                            wv3 = wop[kp].rearrange(
                                "p (i t n) -> p i t n", i=2, t=2)
                            nc.tensor.matmul(
                                vp,
                                lhsT=xop[kp][:, i, :, tb * 128:(tb + 1) * 128],
                                rhs=wv3[:, i, :, :],
                                start=(pi == 0 and kc == 0),
                                stop=(pi == 2 and kc == KC - 1),
                                perf_mode=DR)
                    dst = va4[:, blk:HPG * KB:KB, 0:DH]
                    src = vp.rearrange("p (h d) -> p h d", h=HPG)
                    nc.scalar.copy(dst, src)

                def rope(grp, fin, desc, dve_t3b):
                    # evacuate with the descale; one k-side copy on DVE to
                    # balance the engines
                    sb = [psb.tile([128, 512], BF16, tag=f"qksb{i}", name=f"qksb{i}")
                          for i in range(4)]
                    for i in range(4):
                        if dve_t3b and i == 3:
                            nc.vector.tensor_scalar_mul(sb[i], grp[i], desc)
                        else:
                            nc.scalar.activation(
                                sb[i], grp[i], mybir.ActivationFunctionType.Copy,
                                scale=desc)
                    for ft in range(FT):
                        nc.vector.tensor_mul(
                            fin[ft][:, tsl], sb[ft], cos_sb[:, tsl])
                    for ft in range(FT):
                        t3 = sb[2 + ft]
                        eng = nc.gpsimd if ft == 0 else nc.vector
                        for po in (32, 96):
                            eng.tensor_mul(
                                t3[po:po + 32, :], t3[po:po + 32, :],
                                sin_sb[po:po + 32, tsl])
                            dst = fin[ft][po:po + 32, tsl]
                            nc.vector.tensor_add(dst, dst, t3[po:po + 32, :])

                qg = side(0, (0, 1, 2, 3))
                vproj(0)
                rope(qg, qfin, QDESC, False)
                vproj(1)
                kg = side(4, (4, 5, 0, 1))
                rope(kg, kfin, KDESC, True)
                vproj(2)
                vproj(3)

        # ============ attention + output projection =============
        with tc.tile_pool(name="sc_ps", bufs=2, space="PSUM") as scp, \
             tc.tile_pool(name="dg_ps", bufs=2, space="PSUM") as dgp, \
             tc.tile_pool(name="yt_ps", bufs=2, space="PSUM") as ytp, \
             tc.tile_pool(name="att_sb", bufs=4) as asb, \
             tc.tile_pool(name="dg_sb", bufs=4) as dsb, \
             tc.tile_pool(name="small_sb", bufs=6) as ssb, \
             tc.tile_pool(name="dram_scr", bufs=4, space="DRAM") as dsp, \
             tc.tile_pool(name="out_sb", bufs=3) as osbp:
            w3 = wcs_sb.rearrange("p (t n) -> p t n", t=2)

            def emit_unit(j, h):
                qsl = slice(j * 512, (j + 1) * 512)
                ft, off = h // 2, (h % 2) * 64
                hsl = slice(off, off + 64)
                vsl = lambda b: slice(h * VW + b * 65, h * VW + (b + 1) * 65)
                yp = ytp.tile([128, 512], F32, tag="yt", name="yt")
                # full off-diagonal key blocks, in pairs
                for pi in range(2 * j):
                    sp2 = scp.tile([128, 1024], F32, tag="sc", name="sc")
                    for half in range(2):
                        b = 2 * pi + half
                        nc.tensor.matmul(
                            sp2[:, half * 512:(half + 1) * 512],
                            lhsT=kfin[ft][hsl, b * 128:(b + 1) * 128],
                            rhs=qfin[ft][hsl, qsl],
                            start=True, stop=True)
                    pr2 = asb.tile([128, 1024], BF16, tag="pr", name="pr")
                    nc.scalar.activation(pr2, sp2, EXP)
                    for half in range(2):
                        b = 2 * pi + half
                        nc.tensor.matmul(
                            yp[0:DH + 1, :],
                            lhsT=vaug[:, vsl(b)],
                            rhs=pr2[:, half * 512:(half + 1) * 512],
                            start=(b == 0), stop=False,
                            skip_group_check=True)
                # diagonal band at 128-query granularity.
                # sub-chunk m needs key blocks b = 4j+beta, beta <= m;
                # d := m-beta.  DA: d=0 (masked); DB: d=1 + d=3; DC: d=2.
                DA = dgp.tile([128, 512], F32, tag="dg", name="dg")
                DB = dgp.tile([128, 512], F32, tag="dg", name="dg")
                DC = dgp.tile([128, 512], F32, tag="dg", name="dg")
                qb = j * 512

                def dmm(dst, b, m):
                    nc.tensor.matmul(
                        dst,
                        lhsT=kfin[ft][hsl, b * 128:(b + 1) * 128],
                        rhs=qfin[ft][hsl, qb + m * 128:qb + (m + 1) * 128],
                        start=True, stop=True)

                for m in range(4):
                    dmm(DA[:, m * 128:(m + 1) * 128], 4 * j + m, m)
                for m in range(1, 4):
                    dmm(DB[:, (m - 1) * 128:m * 128], 4 * j + m - 1, m)
                dmm(DB[:, 384:512], 4 * j, 3)
                for m in range(2, 4):
                    dmm(DC[:, (m - 2) * 128:(m - 1) * 128], 4 * j + m - 2, m)
                prA = dsb.tile([128, 512], BF16, tag="prd", name="prd")
                prB = dsb.tile([128, 512], BF16, tag="prd", name="prd")
                prC = dsb.tile([128, 512], BF16, tag="prd", name="prd")
                nc.scalar.activation(prA, DA, EXP)
                nc.scalar.activation(prB, DB, EXP)
                nc.scalar.activation(prC[:, 0:256], DC[:, 0:256], EXP)
                nc.vector.tensor_mul(prA, prA, mask_sb)
                for m in range(4):
                    for beta in range(m + 1):
                        d = m - beta
                        if d == 0:
                            rhs = prA[:, m * 128:(m + 1) * 128]
                        elif d == 1:
                            rhs = prB[:, (m - 1) * 128:m * 128]
                        elif d == 2:
                            rhs = prC[:, (m - 2) * 128:(m - 1) * 128]
                        else:
                            rhs = prB[:, 384:512]
                        nc.tensor.matmul(
                            yp[0:DH + 1, m * 128:(m + 1) * 128],
                            lhsT=vaug[:, vsl(4 * j + beta)],
                            rhs=rhs,
                            start=(j == 0 and beta == 0),
                            stop=(beta == m),
                            skip_group_check=True)
                # normalize: rec = 1/(8*sum pr), broadcast to 64 rows of
                # SBUF via a DRAM round trip (DMA queue has headroom)
                recs = ssb.tile([1, 512], F32, tag="recs", name="recs")
                nc.vector.reciprocal(recs, yp[DH:DH + 1, :])
                rec64 = ssb.tile([64, 512], F32, tag="rec64", name="rec64")
                nc.gpsimd.partition_broadcast(rec64, recs)
                if h % 2 == 0:
                    nc.vector.tensor_mul(
                        y3[0:64, h // 2, qsl], yp[0:DH, :], rec64)
                else:
                    yst = ssb.tile([64, 512], BF16, tag="yst", name="yst")
                    nc.vector.tensor_mul(yst, yp[0:DH, :], rec64)
                    nc.vector.tensor_copy(y3[64:128, h // 2, qsl], yst)

            def emit_cproj_piece(j, mp, tail):
                for s in range(2):
                    ob = osbp.tile([128, C], BF16, tag="ob", name="ob")
                    mi = 4 * j + 2 * mp + s
                    msl = slice(mi * 128, (mi + 1) * 128)
                    for n in range(2):
                        # at the tail the attention pools are idle: rotate
                        # through both PSUM rings for a deeper pipeline
                        if tail and (s + n) % 2 == 1:
                            op = scp.tile([128, 512], F32, tag="sc", name="op")
                        else:
                            op = dgp.tile([128, 512], F32, tag="dg", name="op")
                        for tt in range(2):
                            nc.tensor.matmul(
                                op,
                                lhsT=y3[:, tt, msl],
                                rhs=w3[:, tt, n * 512:(n + 1) * 512],
                                start=(tt == 0), stop=(tt == 1))
                        osl = ob[:, n * 512:(n + 1) * 512]
                        if tail and n == 0:
                            nc.scalar.copy(osl, op)
                        else:
                            nc.vector.tensor_copy(osl, op)
                    nc.sync.dma_start(out[msl, :], ob)

            # heavy chunks interleaved with thin ones so the per-unit
            # serial chains hide under the heavy units' exp work; each
            # pair's c_proj is spread through the NEXT pair's units so
            # its PSUM-evacuation copies hide under exp work too
            pending = []
            for ja, jb in ((3, 0), (2, 1)):
                for h in range(HPG):
                    emit_unit(ja, h)
                    emit_unit(jb, h)
                    if pending:
                        emit_cproj_piece(*pending.pop(0), False)
                pending += [(ja, 0), (ja, 1), (jb, 0), (jb, 1)]
            for j, mp in pending:
                emit_cproj_piece(j, mp, True)


def _host_prep(x, Wq_down, Wk_down, Wv_down, Wq_up_c, Wq_up_e, Wk_up_c,
               Wk_up_e, Wv_up, Wc):
    """Build the per-core input maps."""
    bf = ml_dtypes.bfloat16
    f8 = ml_dtypes.float8_e4m3

    # rope cache, transposed: (32, T)
    inv_freq = 1.0 / (THETA ** (np.arange(0, DHE, 2, dtype=np.float64) / DHE))
    freqs = np.arange(T, dtype=np.float64)[:, None] * inv_freq[None, :]
    emb = np.concatenate((freqs, freqs), axis=-1)  # (T, 32)
    cosT = np.cos(emb).T  # (32, T)
    sinT = np.sin(emb).T

    # signed permutation: rot[2i] = -e[2i+1], rot[2i+1] = e[2i]
    P = np.zeros((DHE, DHE))
    for i in range(DHE // 2):
        P[2 * i, 2 * i + 1] = -1.0
        P[2 * i + 1, 2 * i] = 1.0

    ones32 = np.ones((32, T))
    zeros32 = np.zeros((32, T))
    # T1/T2 rows per head pair: [c(32) | e(32)] x2 -> cos rows [1,cos,1,cos]
    cos4 = np.concatenate([ones32, cosT, ones32, cosT], axis=0)
    # T3a/T3b rows: [0 | rot | 0 | rot] -> sin rows [0,sin,0,sin]
    sin4 = np.concatenate([zeros32, sinT, zeros32, sinT], axis=0)

    # mask for d=0 diagonal blocks: [128, 512] = 4 copies of lower triangle
    kk = np.arange(128)[:, None]
    qq = np.arange(128)[None, :]
    tri = (kk <= qq).astype(np.float64)
    mask_np = np.tile(tri, (1, 4))

    Wq_down = np.asarray(Wq_down, np.float64)
    Wk_down = np.asarray(Wk_down, np.float64)
    Wv_down = np.asarray(Wv_down, np.float64)
    Wq_up_c = np.asarray(Wq_up_c, np.float64)
    Wq_up_e = np.asarray(Wq_up_e, np.float64)
    Wk_up_c = np.asarray(Wk_up_c, np.float64)
    Wk_up_e = np.asarray(Wk_up_e, np.float64)
    Wv_up = np.asarray(Wv_up, np.float64)
    Wc = np.asarray(Wc, np.float64)
    Wq_rot = Wq_up_e @ P.T   # lat -> rot rows (before sin)
    Wk_rot = Wk_up_e @ P.T

    xTs, xRs = [], []
    for b in range(B):
        xT = np.ascontiguousarray(np.asarray(x[b], np.float64).T)
        x8 = xT.astype(f8)
        xTs.append(x8)
        xRs.append((xT - x8.astype(np.float64)).astype(f8))

    in_maps = []
    for core in range(8):
        b, hg = core // HG, core % HG
        # fused q/k down-proj weight [C, 1024]:
        # cols: qT1 qT2 qT3a qT3b kT1 kT2 kT3a kT3b (128 each);
        # T3a/T3b zero-pad the rot rows into partition alignment
        Weff = np.zeros((C, 1024))
        Wveff = np.zeros((C, 256))
        for hh in range(HPG):
            gh = hg * HPG + hh
            lsl = slice(gh * L, (gh + 1) * L)
            po = (hh % 2) * 64   # partition offset within tile
            ftq = (hh // 2) * 128
            rot_col = 256 + ftq + 32 + po  # T3a/T3b, rows 32-63 / 96-127
            Weff[:, ftq + po:ftq + po + 32] = Wq_down[:, lsl] @ Wq_up_c
            Weff[:, ftq + po + 32:ftq + po + 64] = Wq_down[:, lsl] @ Wq_up_e
            Weff[:, rot_col:rot_col + 32] = Wq_down[:, lsl] @ Wq_rot
            Weff[:, 512 + ftq + po:512 + ftq + po + 32] = Wk_down[:, lsl] @ Wk_up_c
            Weff[:, 512 + ftq + po + 32:512 + ftq + po + 64] = Wk_down[:, lsl] @ Wk_up_e
            Weff[:, 512 + rot_col:512 + rot_col + 32] = Wk_down[:, lsl] @ Wk_rot
            Wveff[:, hh * 64:(hh + 1) * 64] = Wv_down[:, lsl] @ Wv_up
        Weff *= WSC
        # DoubleRow pack: dw8[k, p, t*1024+n] = Weff[k*256 + t*128 + p, n]
        dw8 = Weff.reshape(KC, 2, 128, 1024).transpose(0, 2, 1, 3).reshape(
            KC, 128, 2 * 1024)
        Wv64 = Wveff * 64.0
        w8v = Wv64.astype(f8)
        s8v = Wv64 - w8v.astype(np.float64)
        wv8 = w8v.astype(np.float64).reshape(KC, 2, 128, 256).transpose(
            0, 2, 1, 3).reshape(KC, 128, 2 * 256)
        ws8 = s8v.reshape(KC, 2, 128, 256).transpose(
            0, 2, 1, 3).reshape(KC, 128, 2 * 256)
        # c_proj weights, packed over the 256 local y rows:
        # row (p, t) = head (2t + p//64), dim p%64
        wc_slice = Wc[hg * HPG * DH:(hg + 1) * HPG * DH, :]  # (256, C)
        wcsb = np.zeros((128, 2, C))
        for p64 in range(2):
            for t in range(2):
                hh = 2 * t + p64
                wcsb[p64 * 64:(p64 + 1) * 64, t, :] = \
                    wc_slice[hh * DH:(hh + 1) * DH, :]
        in_maps.append({
            "xT8": xTs[b],
            "xr8": xRs[b],
            "dw8": dw8.astype(f8),
            "wv8": wv8.astype(f8),
            "ws8": ws8.astype(f8),
            "cosb": cos4.astype(bf),
            "sinb": sin4.astype(bf),
            "mask": mask_np.astype(bf),
            "wcsb": wcsb.reshape(128, 2 * C).astype(bf),
        })
    return in_maps


LAST_RESULT = {}


def kernel(**inputs):
    inputs = {k: np.asarray(v, dtype=np.float32) for k, v in inputs.items()}
    nc = _build_nc()
    in_maps = _host_prep(**inputs)
    res = run_bass_kernel_spmd(nc, in_maps, core_ids=list(range(8)))
    LAST_RESULT.clear()
    LAST_RESULT.update(
        exec_time_ns=res.exec_time_ns,
        mean_exec_time_ns=res.mean_exec_time_ns,
        profile_json=res.profile_json,
    )
    parts = [r["out"].astype(np.float32) for r in res.results]
    out = np.stack([
        parts[0] + parts[1] + parts[2] + parts[3],
        parts[4] + parts[5] + parts[6] + parts[7],
    ])
    return out.astype(np.float32)


if __name__ == "__main__":
    rng = np.random.default_rng(0)
    ins = {
        "x": rng.standard_normal((B, T, C), dtype=np.float32),
        "Wq_down": rng.standard_normal((C, H * L), dtype=np.float32) * 0.02,
        "Wk_down": rng.standard_normal((C, H * L), dtype=np.float32) * 0.02,
        "Wv_down": rng.standard_normal((C, H * L), dtype=np.float32) * 0.02,
        "Wq_up_c": rng.standard_normal((L, DHE), dtype=np.float32) * 0.02,
        "Wq_up_e": rng.standard_normal((L, DHE), dtype=np.float32) * 0.02,
        "Wk_up_c": rng.standard_normal((L, DHE), dtype=np.float32) * 0.02,
        "Wk_up_e": rng.standard_normal((L, DHE), dtype=np.float32) * 0.02,
        "Wv_up": rng.standard_normal((L, DH), dtype=np.float32) * 0.02,
        "Wc": rng.standard_normal((C, C), dtype=np.float32) * 0.02,
    }
    y = kernel(**ins)
    print(y.shape, y.dtype, float(np.abs(y).mean()))


# revision 5
# speedup vs baseline: 1.0195x; 1.0195x over previous
"""MLA-style attention GPT block on 8 Trainium2 NeuronCores.

Sharding: tensor-parallel over heads x data-parallel over batch.
Core c handles batch b = c // 4 and heads [4*hg, 4*hg+4) with hg = c % 4.
Each core computes a partial c_proj output (2048, 1024) over its 4 heads;
the host sums the 4 partials per batch.

Design notes:
- q/k up-projections (and rope's signed permutation) are folded into the
  down-projection on the host: one fused weight [C, 1024] per core in fp8
  (x64 into e4m3 range; descale folded into the PSUM-evacuation copies,
  the q side also carrying 1/sqrt(DH)).  The rot rows are zero-padded
  into partition alignment so the rope add needs no partition shift.
- v is produced DIRECTLY in attention layout ([keys, dh]) by swapping
  matmul operands (x chunk stationary, fused Wv moving); fp8 precision
  is recovered with residual passes: 64*v = (x8+r8)'w8 + x8's8.  The
  x64 descale rides the softmax-denominator ones row.
- Down-proj and v-proj run as fp8 DoubleRow matmuls (K=256/instruction,
  0.5 cycles/row); scores/att@V/c_proj run in bf16 (the value path does
  not tolerate fp8: relative quantization error survives contractions).
- Causal structure: per 512-query chunk, off-diagonal 128-key blocks are
  computed in [128,1024] PSUM pairs (one exp each); the diagonal band is
  split at 128-query granularity into 10 small matmuls in 3 PSUM tiles
  (3 exps), leaving one [128,512] mask multiply per (head, chunk) on DVE.
- Softmax normalization per (head, q-chunk): the denominator row comes
  free from a 64.0-row in the att@V matmul, is reciprocaled on DVE and
  broadcast across partitions by gpsimd.partition_broadcast; odd heads
  stage the normalize product and use a shifted TensorCopy into the
  c_proj layout (TensorTensor ops require same-start partitions).
- q-chunks are processed in interleaved pairs (3,0), (2,1) so thin
  chunks hide under heavy ones; each pair's c_proj is spread through the
  next pair's units, and the tail c_proj rotates through both PSUM rings.
"""

import sys

sys.path.insert(0, "/opt/trn_rl_repo")

import ml_dtypes
import numpy as np

import concourse.bass as bass
import concourse.tile as tile
from concourse import bacc
from concourse import mybir
from concourse.bass_utils import run_bass_kernel_spmd

B, T, C = 2, 2048, 1024
H, L = 16, 64
DH = 64
DHE = 32
THETA = 10000.0

HG = 4    # head-groups (cores per batch)
HPG = H // HG   # heads per core = 4
FT = 2    # head-pair tiles

KC = 4    # down-proj contraction chunks of 256 (DoubleRow)
TC = 4    # chunks of 512 along T
QB = 4    # query chunks of 512
KB = 16   # key blocks of 128
VW = 1040  # vaug stride per head (16 blocks x 65)

WSC = 64.0                  # fp8 range rescale for the fused q/k weights
QDESC = 1.0 / (WSC * 8.0)   # q descale: weight rescale + 1/sqrt(DH)
KDESC = 1.0 / WSC

F32 = mybir.dt.float32
F32R = mybir.dt.float32r
BF16 = mybir.dt.bfloat16
FP8 = mybir.dt.float8e4
DR = mybir.MatmulPerfMode.DoubleRow
EXP = mybir.ActivationFunctionType.Exp

_NC_CACHE = {}


def _build_nc():
    if "nc" in _NC_CACHE:
        return _NC_CACHE["nc"]
    nc = bacc.Bacc("TRN2", target_bir_lowering=False)

    xT8 = nc.dram_tensor("xT8", [C, T], FP8, kind="ExternalInput")
    dw8 = nc.dram_tensor("dw8", [KC, 128, 2 * 1024], FP8, kind="ExternalInput")
    xr8 = nc.dram_tensor("xr8", [C, T], FP8, kind="ExternalInput")
    wv8 = nc.dram_tensor("wv8", [KC, 128, 2 * 256], FP8, kind="ExternalInput")
    ws8 = nc.dram_tensor("ws8", [KC, 128, 2 * 256], FP8, kind="ExternalInput")
    cosb = nc.dram_tensor("cosb", [128, T], BF16, kind="ExternalInput")
    sinb = nc.dram_tensor("sinb", [128, T], BF16, kind="ExternalInput")
    mask = nc.dram_tensor("mask", [128, 512], BF16, kind="ExternalInput")
    wcsb = nc.dram_tensor("wcsb", [128, 2 * C], BF16, kind="ExternalInput")
    out = nc.dram_tensor("out", [T, C], BF16, kind="ExternalOutput")

    with tile.TileContext(nc) as tc:
        _emit(nc, tc, xT8, xr8, dw8, wv8, ws8, cosb, sinb, mask, wcsb, out)
    nc.compile()

    _NC_CACHE["nc"] = nc
    return nc


def _emit(nc, tc, xT8, xr8, dw8, wv8, ws8, cosb, sinb, mask, wcsb, out):
    from contextlib import ExitStack

    ctx = ExitStack()
    with ctx:
        consts = ctx.enter_context(tc.tile_pool(name="consts", bufs=1))
        persist = ctx.enter_context(tc.tile_pool(name="persist", bufs=1))

        # ---- persistent activations ----
        qfin = [persist.tile([128, T], BF16, tag=f"qfin{t}", name=f"qfin{t}") for t in range(FT)]
        kfin = [persist.tile([128, T], BF16, tag=f"kfin{t}", name=f"kfin{t}") for t in range(FT)]
        vaug = persist.tile([128, HPG * VW], BF16, tag="vaug", name="vaug")
        ycoreb = persist.tile([128, 2 * T], BF16, tag="ycoreb", name="ycoreb")
        y3 = ycoreb.rearrange("p (t x) -> p t x", t=2)
        va4 = vaug.rearrange("p (h b c) -> p (h b) c", h=HPG, c=65)
        nc.gpsimd.memset(va4[:, :, DH], 64.0)

        # ================= projection phase =================
        with tc.tile_pool(name="dwp", bufs=1) as dwp, \
             tc.tile_pool(name="qk_ps", bufs=1, space="PSUM") as qkps, \
             tc.tile_pool(name="v_ps", bufs=2, space="PSUM") as vps, \
             tc.tile_pool(name="proj_sb", bufs=2) as psb, \
             tc.tile_pool(name="xpieces", bufs=3) as xpool:
            # critical-path-first DMA order: x chunk 0 + weights, then consts
            def load_x(tsl):
                xps = []
                for kp in range(2):
                    xp = xpool.tile([128, 4 * 512], FP8, tag=f"xp{kp}", name="xp")
                    src = xT8[kp * 512:(kp + 1) * 512, tsl]
                    nc.sync.dma_start(
                        xp.rearrange("p (i t x) -> p i t x", i=2, t=2),
                        src.rearrange("(i t p) x -> p i t x", i=2, t=2))
                    xps.append(xp)
                for kp in range(2):
                    xr = xpool.tile([128, 4 * 512], FP8, tag=f"xr{kp}", name="xr")
                    src = xr8[kp * 512:(kp + 1) * 512, tsl]
                    nc.sync.dma_start(
                        xr.rearrange("p (i t x) -> p i t x", i=2, t=2),
                        src.rearrange("(i t p) x -> p i t x", i=2, t=2))
                    xps.append(xr)
                return xps

            # interleave so matmul (g, kc) can start as soon as its
            # operands land: xp[kp0], dw0, dw1, xp[kp1], dw2, dw3, wv
            xps = []
            dwt = []
            for kp in range(2):
                xp = xpool.tile([128, 4 * 512], FP8, tag=f"xp{kp}", name="xp")
                src = xT8[kp * 512:(kp + 1) * 512, 0:512]
                nc.sync.dma_start(
                    xp.rearrange("p (i t x) -> p i t x", i=2, t=2),
                    src.rearrange("(i t p) x -> p i t x", i=2, t=2))
                xps.append(xp)
                for k in (2 * kp, 2 * kp + 1):
                    dw = dwp.tile([128, 2 * 1024], FP8, tag=f"dw{k}", name=f"dw{k}")
                    nc.sync.dma_start(dw, dw8[k, :, :])
                    dwt.append(dw)
            for kp in range(2):
                xr = xpool.tile([128, 4 * 512], FP8, tag=f"xr{kp}", name="xr")
                srcr = xr8[kp * 512:(kp + 1) * 512, 0:512]
                nc.sync.dma_start(
                    xr.rearrange("p (i t x) -> p i t x", i=2, t=2),
                    srcr.rearrange("(i t p) x -> p i t x", i=2, t=2))
                xps.append(xr)
            wvt = []
            wst = []
            for kp in range(2):
                wv = dwp.tile([128, 2 * 512], FP8, tag=f"wv{kp}", name=f"wv{kp}")
                nc.sync.dma_start(
                    wv.rearrange("p (i r) -> p i r", i=2),
                    wv8[2 * kp:2 * kp + 2, :, :].rearrange("i p r -> p i r"))
                wvt.append(wv)
                ws = dwp.tile([128, 2 * 512], FP8, tag=f"ws{kp}", name=f"ws{kp}")
                nc.sync.dma_start(
                    ws.rearrange("p (i r) -> p i r", i=2),
                    ws8[2 * kp:2 * kp + 2, :, :].rearrange("i p r -> p i r"))
                wst.append(ws)
            # rope tables (shared q/k; descale rides the evacuation copies)
            cos_sb = consts.tile([128, T], BF16, tag="cosb", name="cos_sb")
            sin_sb = consts.tile([128, T], BF16, tag="sinb", name="sin_sb")
            tables = ((cos_sb, cosb), (sin_sb, sinb))

            def load_tables(tsl):
                for dst, src in tables:
                    nc.sync.dma_start(dst[:, tsl], src[:, tsl])

            load_tables(slice(0, 512))
            xps_pre = [load_x(slice(512, 1024)), load_x(slice(1024, 1536))]
            load_tables(slice(512, 1024))
            load_tables(slice(1024, 1536))
            mask_sb = consts.tile([128, 512], BF16, tag="mask", name="mask_sb")
            wcs_sb = consts.tile([128, 2 * C], BF16, tag="wcsb", name="wcs_sb")
            nc.sync.dma_start(mask_sb, mask[:, :])
            nc.sync.dma_start(wcs_sb, wcsb[:, :])

            for t in range(TC):
                tsl = slice(t * 512, (t + 1) * 512)
                if t in (1, 2):
                    xps = xps_pre[t - 1]
                elif t > 0:
                    xps = load_x(tsl)
                    load_tables(tsl)
                xv = [xp.rearrange("p (i t x) -> p i t x", i=2, t=2) for xp in xps[:2]]

                # 4 accumulating groups per side: T1 T2 T3a T3b; k side
                # reuses the q banks through the bufs=1 ring.
                def side(base, tags):
                    grp = [qkps.tile([128, 512], F32, tag=f"g{i}", name=f"g{i}")
                           for i in tags]
                    for g in range(4):
                        for kc in range(KC):
                            kp, i = kc // 2, kc % 2
                            dw3 = dwt[kc].rearrange("p (t n) -> p t n", t=2)
                            nc.tensor.matmul(
                                grp[g],
                                lhsT=dw3[:, :, (base + g) * 128:(base + g + 1) * 128],
                                rhs=xv[kp][:, i, :, :],
                                start=(kc == 0), stop=(kc == KC - 1),
                                perf_mode=DR)
                    return grp

                def vproj(tb):
                    # 64*v = (x8 + r8)' w8 + x8' s8  (fp8 DoubleRow passes)
                    blk = 4 * t + tb
                    vp = vps.tile([128, 256], F32, tag="vp", name="vp")
                    xrv = [xps[2].rearrange("p (i t x) -> p i t x", i=2, t=2),
                           xps[3].rearrange("p (i t x) -> p i t x", i=2, t=2)]
                    for pi, (xop, wop) in enumerate(
                            ((xv, wvt), (xrv, wvt), (xv, wst))):
                        for kc in range(KC):
                            kp, i = kc // 2, kc % 2
                            wv3 = wop[kp].rearrange(
                                "p (i t n) -> p i t n", i=2, t=2)
                            nc.tensor.matmul(
                                vp,
                                lhsT=xop[kp][:, i, :, tb * 128:(tb + 1) * 128],
                                rhs=wv3[:, i, :, :],
                                start=(pi == 0 and kc == 0),
                                stop=(pi == 2 and kc == KC - 1),
                                perf_mode=DR)
                    dst = va4[:, blk:HPG * KB:KB, 0:DH]
                    src = vp.rearrange("p (h d) -> p h d", h=HPG)
                    nc.scalar.copy(dst, src)

                def rope(grp, fin, desc, dve_t3b):
                    # evacuate with the descale; one k-side copy on DVE to
                    # balance the engines
                    sb = [psb.tile([128, 512], BF16, tag=f"qksb{i}", name=f"qksb{i}")
                          for i in range(4)]
                    for i in range(4):
                        if dve_t3b and i == 3:
                            nc.vector.tensor_scalar_mul(sb[i], grp[i], desc)
                        else:
                            nc.scalar.activation(
                                sb[i], grp[i], mybir.ActivationFunctionType.Copy,
                                scale=desc)
                    for ft in range(FT):
                        nc.vector.tensor_mul(
                            fin[ft][:, tsl], sb[ft], cos_sb[:, tsl])
                    for ft in range(FT):
                        t3 = sb[2 + ft]
                        eng = nc.gpsimd if ft == 0 else nc.vector
                        for po in (32, 96):
                            eng.tensor_mul(
                                t3[po:po + 32, :], t3[po:po + 32, :],
                                sin_sb[po:po + 32, tsl])
                            dst = fin[ft][po:po + 32, tsl]
                            nc.vector.tensor_add(dst, dst, t3[po:po + 32, :])

                qg = side(0, (0, 1, 2, 3))
                vproj(0)
                rope(qg, qfin, QDESC, False)
                vproj(1)
                kg = side(4, (4, 5, 0, 1))
                rope(kg, kfin, KDESC, True)
                vproj(2)
                vproj(3)

        # ============ attention + output projection =============
        with tc.tile_pool(name="sc_ps", bufs=2, space="PSUM") as scp, \
             tc.tile_pool(name="dg_ps", bufs=2, space="PSUM") as dgp, \
             tc.tile_pool(name="yt_ps", bufs=2, space="PSUM") as ytp, \
             tc.tile_pool(name="att_sb", bufs=4) as asb, \
             tc.tile_pool(name="dg_sb", bufs=4) as dsb, \
             tc.tile_pool(name="small_sb", bufs=6) as ssb, \
             tc.tile_pool(name="dram_scr", bufs=4, space="DRAM") as dsp, \
             tc.tile_pool(name="out_sb", bufs=3) as osbp:
            w3 = wcs_sb.rearrange("p (t n) -> p t n", t=2)

            def emit_unit(j, h):
                qsl = slice(j * 512, (j + 1) * 512)
                ft, off = h // 2, (h % 2) * 64
                hsl = slice(off, off + 64)
                vsl = lambda b: slice(h * VW + b * 65, h * VW + (b + 1) * 65)
                yp = ytp.tile([128, 512], F32, tag="yt", name="yt")
                # full off-diagonal key blocks, in pairs
                for pi in range(2 * j):
                    sp2 = scp.tile([128, 1024], F32, tag="sc", name="sc")
                    for half in range(2):
                        b = 2 * pi + half
                        nc.tensor.matmul(
                            sp2[:, half * 512:(half + 1) * 512],
                            lhsT=kfin[ft][hsl, b * 128:(b + 1) * 128],
                            rhs=qfin[ft][hsl, qsl],
                            start=True, stop=True)
                    pr2 = asb.tile([128, 1024], BF16, tag="pr", name="pr")
                    nc.scalar.activation(pr2, sp2, EXP)
                    for half in range(2):
                        b = 2 * pi + half
                        nc.tensor.matmul(
                            yp[0:DH + 1, :],
                            lhsT=vaug[:, vsl(b)],
                            rhs=pr2[:, half * 512:(half + 1) * 512],
                            start=(b == 0), stop=False,
                            skip_group_check=True)
                # diagonal band at 128-query granularity.
                # sub-chunk m needs key blocks b = 4j+beta, beta <= m;
                # d := m-beta.  DA: d=0 (masked); DB: d=1 + d=3; DC: d=2.
                DA = dgp.tile([128, 512], F32, tag="dg", name="dg")
                DB = dgp.tile([128, 512], F32, tag="dg", name="dg")
                DC = dgp.tile([128, 512], F32, tag="dg", name="dg")
                qb = j * 512

                def dmm(dst, b, m):
                    nc.tensor.matmul(
                        dst,
                        lhsT=kfin[ft][hsl, b * 128:(b + 1) * 128],
                        rhs=qfin[ft][hsl, qb + m * 128:qb + (m + 1) * 128],
                        start=True, stop=True)

                for m in range(4):
                    dmm(DA[:, m * 128:(m + 1) * 128], 4 * j + m, m)
                for m in range(1, 4):
                    dmm(DB[:, (m - 1) * 128:m * 128], 4 * j + m - 1, m)
                dmm(DB[:, 384:512], 4 * j, 3)
                for m in range(2, 4):
                    dmm(DC[:, (m - 2) * 128:(m - 1) * 128], 4 * j + m - 2, m)
                prA = dsb.tile([128, 512], BF16, tag="prd", name="prd")
                prB = dsb.tile([128, 512], BF16, tag="prd", name="prd")
                prC = dsb.tile([128, 512], BF16, tag="prd", name="prd")
                nc.scalar.activation(prA, DA, EXP)
                nc.scalar.activation(prB, DB, EXP)
                nc.scalar.activation(prC[:, 0:256], DC[:, 0:256], EXP)
                nc.vector.tensor_mul(prA, prA, mask_sb)
                for m in range(4):
                    for beta in range(m + 1):
                        d = m - beta
                        if d == 0:
                            rhs = prA[:, m * 128:(m + 1) * 128]
                        elif d == 1:
                            rhs = prB[:, (m - 1) * 128:m * 128]
                        elif d == 2:
                            rhs = prC[:, (m - 2) * 128:(m - 1) * 128]
                        else:
                            rhs = prB[:, 384:512]
                        nc.tensor.matmul(
                            yp[0:DH + 1, m * 128:(m + 1) * 128],
                            lhsT=vaug[:, vsl(4 * j + beta)],
                            rhs=rhs,
                            start=(j == 0 and beta == 0),
                            stop=(beta == m),
                            skip_group_check=True)
                # normalize: rec = 1/(8*sum pr), broadcast to 64 rows of
                # SBUF via a DRAM round trip (DMA queue has headroom)
                recs = ssb.tile([1, 512], F32, tag="recs", name="recs")
                nc.vector.reciprocal(recs, yp[DH:DH + 1, :])
                rec64 = ssb.tile([64, 512], F32, tag="rec64", name="rec64")
                nc.gpsimd.partition_broadcast(rec64, recs)
                if h % 2 == 0:
                    nc.vector.tensor_mul(
                        y3[0:64, h // 2, qsl], yp[0:DH, :], rec64)
                else:
                    yst = ssb.tile([64, 512], BF16, tag="yst", name="yst")
                    nc.vector.tensor_mul(yst, yp[0:DH, :], rec64)
                    nc.vector.tensor_copy(y3[64:128, h // 2, qsl], yst)

            def emit_cproj_piece(j, mp, tail):
                for s in range(2):
                    ob = osbp.tile([128, C], BF16, tag="ob", name="ob")
                    mi = 4 * j + 2 * mp + s
                    msl = slice(mi * 128, (mi + 1) * 128)
                    for n in range(2):
                        # at the tail the attention pools are idle: rotate
                        # through both PSUM rings for a deeper pipeline
                        if tail and (s + n) % 2 == 1:
                            op = scp.tile([128, 512], F32, tag="sc", name="op")
                        else:
                            op = dgp.tile([128, 512], F32, tag="dg", name="op")
                        for tt in range(2):
                            nc.tensor.matmul(
                                op,
                                lhsT=y3[:, tt, msl],
                                rhs=w3[:, tt, n * 512:(n + 1) * 512],
                                start=(tt == 0), stop=(tt == 1))
                        osl = ob[:, n * 512:(n + 1) * 512]
                        if tail and n == 0:
                            nc.scalar.copy(osl, op)
                        else:
                            nc.vector.tensor_copy(osl, op)
                    nc.sync.dma_start(out[msl, :], ob)

            # heavy chunks interleaved with thin ones (thin first, so
            # its short serial chain drains under the heavy unit's exp
            # stream); each pair's c_proj is spread through the NEXT
            # pair's units so its copies hide under exp work too
            pending = []
            for ja, jb in ((3, 0), (2, 1)):
                for h in range(HPG):
                    emit_unit(jb, h)
                    emit_unit(ja, h)
                    if pending:
                        emit_cproj_piece(*pending.pop(0), False)
                pending += [(ja, 0), (ja, 1), (jb, 0), (jb, 1)]
            for j, mp in pending:
                emit_cproj_piece(j, mp, True)


def _host_prep(x, Wq_down, Wk_down, Wv_down, Wq_up_c, Wq_up_e, Wk_up_c,
               Wk_up_e, Wv_up, Wc):
    """Build the per-core input maps."""
    bf = ml_dtypes.bfloat16
    f8 = ml_dtypes.float8_e4m3

    # rope cache, transposed: (32, T)
    inv_freq = 1.0 / (THETA ** (np.arange(0, DHE, 2, dtype=np.float64) / DHE))
    freqs = np.arange(T, dtype=np.float64)[:, None] * inv_freq[None, :]
    emb = np.concatenate((freqs, freqs), axis=-1)  # (T, 32)
    cosT = np.cos(emb).T  # (32, T)
    sinT = np.sin(emb).T

    # signed permutation: rot[2i] = -e[2i+1], rot[2i+1] = e[2i]
    P = np.zeros((DHE, DHE))
    for i in range(DHE // 2):
        P[2 * i, 2 * i + 1] = -1.0
        P[2 * i + 1, 2 * i] = 1.0

    ones32 = np.ones((32, T))
    zeros32 = np.zeros((32, T))
    # T1/T2 rows per head pair: [c(32) | e(32)] x2 -> cos rows [1,cos,1,cos]
    cos4 = np.concatenate([ones32, cosT, ones32, cosT], axis=0)
    # T3a/T3b rows: [0 | rot | 0 | rot] -> sin rows [0,sin,0,sin]
    sin4 = np.concatenate([zeros32, sinT, zeros32, sinT], axis=0)

    # mask for d=0 diagonal blocks: [128, 512] = 4 copies of lower triangle
    kk = np.arange(128)[:, None]
    qq = np.arange(128)[None, :]
    tri = (kk <= qq).astype(np.float64)
    mask_np = np.tile(tri, (1, 4))

    Wq_down = np.asarray(Wq_down, np.float64)
    Wk_down = np.asarray(Wk_down, np.float64)
    Wv_down = np.asarray(Wv_down, np.float64)
    Wq_up_c = np.asarray(Wq_up_c, np.float64)
    Wq_up_e = np.asarray(Wq_up_e, np.float64)
    Wk_up_c = np.asarray(Wk_up_c, np.float64)
    Wk_up_e = np.asarray(Wk_up_e, np.float64)
    Wv_up = np.asarray(Wv_up, np.float64)
    Wc = np.asarray(Wc, np.float64)
    Wq_rot = Wq_up_e @ P.T   # lat -> rot rows (before sin)
    Wk_rot = Wk_up_e @ P.T

    xTs, xRs = [], []
    for b in range(B):
        xT = np.ascontiguousarray(np.asarray(x[b], np.float64).T)
        x8 = xT.astype(f8)
        xTs.append(x8)
        xRs.append((xT - x8.astype(np.float64)).astype(f8))

    in_maps = []
    for core in range(8):
        b, hg = core // HG, core % HG
        # fused q/k down-proj weight [C, 1024]:
        # cols: qT1 qT2 qT3a qT3b kT1 kT2 kT3a kT3b (128 each);
        # T3a/T3b zero-pad the rot rows into partition alignment
        Weff = np.zeros((C, 1024))
        Wveff = np.zeros((C, 256))
        for hh in range(HPG):
            gh = hg * HPG + hh
            lsl = slice(gh * L, (gh + 1) * L)
            po = (hh % 2) * 64   # partition offset within tile
            ftq = (hh // 2) * 128
            rot_col = 256 + ftq + 32 + po  # T3a/T3b, rows 32-63 / 96-127
            Weff[:, ftq + po:ftq + po + 32] = Wq_down[:, lsl] @ Wq_up_c
            Weff[:, ftq + po + 32:ftq + po + 64] = Wq_down[:, lsl] @ Wq_up_e
            Weff[:, rot_col:rot_col + 32] = Wq_down[:, lsl] @ Wq_rot
            Weff[:, 512 + ftq + po:512 + ftq + po + 32] = Wk_down[:, lsl] @ Wk_up_c
            Weff[:, 512 + ftq + po + 32:512 + ftq + po + 64] = Wk_down[:, lsl] @ Wk_up_e
            Weff[:, 512 + rot_col:512 + rot_col + 32] = Wk_down[:, lsl] @ Wk_rot
            Wveff[:, hh * 64:(hh + 1) * 64] = Wv_down[:, lsl] @ Wv_up
        Weff *= WSC
        # DoubleRow pack: dw8[k, p, t*1024+n] = Weff[k*256 + t*128 + p, n]
        dw8 = Weff.reshape(KC, 2, 128, 1024).transpose(0, 2, 1, 3).reshape(
            KC, 128, 2 * 1024)
        Wv64 = Wveff * 64.0
        w8v = Wv64.astype(f8)
        s8v = Wv64 - w8v.astype(np.float64)
        wv8 = w8v.astype(np.float64).reshape(KC, 2, 128, 256).transpose(
            0, 2, 1, 3).reshape(KC, 128, 2 * 256)
        ws8 = s8v.reshape(KC, 2, 128, 256).transpose(
            0, 2, 1, 3).reshape(KC, 128, 2 * 256)
        # c_proj weights, packed over the 256 local y rows:
        # row (p, t) = head (2t + p//64), dim p%64
        wc_slice = Wc[hg * HPG * DH:(hg + 1) * HPG * DH, :]  # (256, C)
        wcsb = np.zeros((128, 2, C))
        for p64 in range(2):
            for t in range(2):
                hh = 2 * t + p64
                wcsb[p64 * 64:(p64 + 1) * 64, t, :] = \
                    wc_slice[hh * DH:(hh + 1) * DH, :]
        in_maps.append({
            "xT8": xTs[b],
            "xr8": xRs[b],
            "dw8": dw8.astype(f8),
            "wv8": wv8.astype(f8),
            "ws8": ws8.astype(f8),
            "cosb": cos4.astype(bf),
            "sinb": sin4.astype(bf),
            "mask": mask_np.astype(bf),
            "wcsb": wcsb.reshape(128, 2 * C).astype(bf),
        })
    return in_maps


LAST_RESULT = {}


def kernel(**inputs):
    inputs = {k: np.asarray(v, dtype=np.float32) for k, v in inputs.items()}
    nc = _build_nc()
    in_maps = _host_prep(**inputs)
    res = run_bass_kernel_spmd(nc, in_maps, core_ids=list(range(8)))
    LAST_RESULT.clear()
    LAST_RESULT.update(
        exec_time_ns=res.exec_time_ns,
        mean_exec_time_ns=res.mean_exec_time_ns,
        profile_json=res.profile_json,
    )
    parts = [r["out"].astype(np.float32) for r in res.results]
    out = np.stack([
        parts[0] + parts[1] + parts[2] + parts[3],
        parts[4] + parts[5] + parts[6] + parts[7],
    ])
    return out.astype(np.float32)


if __name__ == "__main__":
    rng = np.random.default_rng(0)
    ins = {
        "x": rng.standard_normal((B, T, C), dtype=np.float32),
        "Wq_down": rng.standard_normal((C, H * L), dtype=np.float32) * 0.02,
        "Wk_down": rng.standard_normal((C, H * L), dtype=np.float32) * 0.02,
        "Wv_down": rng.standard_normal((C, H * L), dtype=np.float32) * 0.02,
        "Wq_up_c": rng.standard_normal((L, DHE), dtype=np.float32) * 0.02,
        "Wq_up_e": rng.standard_normal((L, DHE), dtype=np.float32) * 0.02,
        "Wk_up_c": rng.standard_normal((L, DHE), dtype=np.float32) * 0.02,
        "Wk_up_e": rng.standard_normal((L, DHE), dtype=np.float32) * 0.02,
        "Wv_up": rng.standard_normal((L, DH), dtype=np.float32) * 0.02,
        "Wc": rng.standard_normal((C, C), dtype=np.float32) * 0.02,
    }
    y = kernel(**ins)
    print(y.shape, y.dtype, float(np.abs(y).mean()))


# revision 6
# speedup vs baseline: 1.0468x; 1.0268x over previous
"""MLA-style attention GPT block on 8 Trainium2 NeuronCores.

Sharding: tensor-parallel over heads x data-parallel over batch.
Core c handles batch b = c // 4 and heads [4*hg, 4*hg+4) with hg = c % 4.
Each core computes a partial c_proj output (2048, 1024) over its 4 heads;
the host sums the 4 partials per batch.

Design notes:
- q/k up-projections (and rope's signed permutation) are folded into the
  down-projection on the host: one fused weight [C, 1024] per core in fp8
  (x64 into e4m3 range; descale folded into the PSUM-evacuation copies,
  the q side also carrying 1/sqrt(DH)).  The rot rows are zero-padded
  into partition alignment so the rope add needs no partition shift.
- v is produced DIRECTLY in attention layout ([keys, dh]) by swapping
  matmul operands (x chunk stationary, fused Wv moving); fp8 precision
  is recovered with residual passes: 64*v = (x8+r8)'w8 + x8's8.  The
  x64 descale rides the softmax-denominator ones row.
- Down-proj and v-proj run as fp8 DoubleRow matmuls (K=256/instruction,
  0.5 cycles/row); scores/att@V/c_proj run in bf16 (the value path does
  not tolerate fp8: relative quantization error survives contractions).
- Causal structure: per 512-query chunk, off-diagonal 128-key blocks are
  computed in [128,1024] PSUM pairs (one exp each); the diagonal band is
  split at 128-query granularity into 10 small matmuls in 3 PSUM tiles
  (3 exps), leaving one [128,512] mask multiply per (head, chunk) on DVE.
- Softmax normalization per (head, q-chunk): the denominator row comes
  free from a 64.0-row in the att@V matmul, is reciprocaled on DVE and
  broadcast across partitions by gpsimd.partition_broadcast; odd heads
  stage the normalize product and use a shifted TensorCopy into the
  c_proj layout (TensorTensor ops require same-start partitions).
- q-chunks are processed in interleaved pairs (3,0), (2,1) so thin
  chunks hide under heavy ones; each pair's c_proj is spread through the
  next pair's units, and the tail c_proj rotates through both PSUM rings.
"""

import sys

sys.path.insert(0, "/opt/trn_rl_repo")

import ml_dtypes
import numpy as np

import concourse.bass as bass
import concourse.tile as tile
from concourse import bacc
from concourse import mybir
from concourse.bass_utils import run_bass_kernel_spmd

B, T, C = 2, 2048, 1024
H, L = 16, 64
DH = 64
DHE = 32
THETA = 10000.0

HG = 4    # head-groups (cores per batch)
HPG = H // HG   # heads per core = 4
FT = 2    # head-pair tiles

KC = 4    # down-proj contraction chunks of 256 (DoubleRow)
TC = 4    # chunks of 512 along T
QB = 4    # query chunks of 512
KB = 16   # key blocks of 128
VW = 1040  # vaug stride per head (16 blocks x 65)

WSC = 64.0                  # fp8 range rescale for the fused q/k weights
QDESC = 1.0 / (WSC * 8.0)   # q descale: weight rescale + 1/sqrt(DH)
KDESC = 1.0 / WSC

F32 = mybir.dt.float32
F32R = mybir.dt.float32r
BF16 = mybir.dt.bfloat16
FP8 = mybir.dt.float8e4
DR = mybir.MatmulPerfMode.DoubleRow
EXP = mybir.ActivationFunctionType.Exp

_NC_CACHE = {}


def _build_nc():
    if "nc" in _NC_CACHE:
        return _NC_CACHE["nc"]
    nc = bacc.Bacc("TRN2", target_bir_lowering=False)

    xT8 = nc.dram_tensor("xT8", [C, T], FP8, kind="ExternalInput")
    dw8 = nc.dram_tensor("dw8", [KC, 128, 2 * 1024], FP8, kind="ExternalInput")
    xr8 = nc.dram_tensor("xr8", [C, T], FP8, kind="ExternalInput")
    wv8 = nc.dram_tensor("wv8", [KC, 128, 2 * 256], FP8, kind="ExternalInput")
    ws8 = nc.dram_tensor("ws8", [KC, 128, 2 * 256], FP8, kind="ExternalInput")
    cosb = nc.dram_tensor("cosb", [128, T], BF16, kind="ExternalInput")
    sinb = nc.dram_tensor("sinb", [128, T], BF16, kind="ExternalInput")
    mask = nc.dram_tensor("mask", [128, 512], BF16, kind="ExternalInput")
    wcsb = nc.dram_tensor("wcsb", [128, 2 * C], BF16, kind="ExternalInput")
    out = nc.dram_tensor("out", [T, C], BF16, kind="ExternalOutput")

    with tile.TileContext(nc) as tc:
        _emit(nc, tc, xT8, xr8, dw8, wv8, ws8, cosb, sinb, mask, wcsb, out)
    nc.compile()

    _NC_CACHE["nc"] = nc
    return nc


def _emit(nc, tc, xT8, xr8, dw8, wv8, ws8, cosb, sinb, mask, wcsb, out):
    from contextlib import ExitStack

    ctx = ExitStack()
    with ctx:
        consts = ctx.enter_context(tc.tile_pool(name="consts", bufs=1))
        persist = ctx.enter_context(tc.tile_pool(name="persist", bufs=1))

        # ---- persistent activations ----
        qfin = [persist.tile([128, T], BF16, tag=f"qfin{t}", name=f"qfin{t}") for t in range(FT)]
        kfin = [persist.tile([128, T], BF16, tag=f"kfin{t}", name=f"kfin{t}") for t in range(FT)]
        vaug = persist.tile([128, HPG * VW], BF16, tag="vaug", name="vaug")
        ycoreb = persist.tile([128, 2 * T], BF16, tag="ycoreb", name="ycoreb")
        y3 = ycoreb.rearrange("p (t x) -> p t x", t=2)
        va4 = vaug.rearrange("p (h b c) -> p (h b) c", h=HPG, c=65)
        nc.gpsimd.memset(va4[:, :, DH], 64.0)

        # ================= projection phase =================
        with tc.tile_pool(name="dwp", bufs=1) as dwp, \
             tc.tile_pool(name="qk_ps", bufs=1, space="PSUM") as qkps, \
             tc.tile_pool(name="v_ps", bufs=2, space="PSUM") as vps, \
             tc.tile_pool(name="proj_sb", bufs=2) as psb, \
             tc.tile_pool(name="xpieces", bufs=3) as xpool:
            # critical-path-first DMA order: x chunk 0 + weights, then consts
            def load_x(tsl):
                xps = []
                for kp in range(2):
                    xp = xpool.tile([128, 4 * 512], FP8, tag=f"xp{kp}", name="xp")
                    src = xT8[kp * 512:(kp + 1) * 512, tsl]
                    nc.sync.dma_start(
                        xp.rearrange("p (i t x) -> p i t x", i=2, t=2),
                        src.rearrange("(i t p) x -> p i t x", i=2, t=2))
                    xps.append(xp)
                for kp in range(2):
                    xr = xpool.tile([128, 4 * 512], FP8, tag=f"xr{kp}", name="xr")
                    src = xr8[kp * 512:(kp + 1) * 512, tsl]
                    nc.sync.dma_start(
                        xr.rearrange("p (i t x) -> p i t x", i=2, t=2),
                        src.rearrange("(i t p) x -> p i t x", i=2, t=2))
                    xps.append(xr)
                return xps

            # interleave so matmul (g, kc) can start as soon as its
            # operands land: xp[kp0], dw0, dw1, xp[kp1], dw2, dw3, wv
            xps = []
            dwt = []
            for kp in range(2):
                xp = xpool.tile([128, 4 * 512], FP8, tag=f"xp{kp}", name="xp")
                src = xT8[kp * 512:(kp + 1) * 512, 0:512]
                nc.sync.dma_start(
                    xp.rearrange("p (i t x) -> p i t x", i=2, t=2),
                    src.rearrange("(i t p) x -> p i t x", i=2, t=2))
                xps.append(xp)
                for k in (2 * kp, 2 * kp + 1):
                    dw = dwp.tile([128, 2 * 1024], FP8, tag=f"dw{k}", name=f"dw{k}")
                    nc.sync.dma_start(dw, dw8[k, :, :])
                    dwt.append(dw)
            for kp in range(2):
                xr = xpool.tile([128, 4 * 512], FP8, tag=f"xr{kp}", name="xr")
                srcr = xr8[kp * 512:(kp + 1) * 512, 0:512]
                nc.sync.dma_start(
                    xr.rearrange("p (i t x) -> p i t x", i=2, t=2),
                    srcr.rearrange("(i t p) x -> p i t x", i=2, t=2))
                xps.append(xr)
            wvt = []
            wst = []
            for kp in range(2):
                wv = dwp.tile([128, 2 * 512], FP8, tag=f"wv{kp}", name=f"wv{kp}")
                nc.sync.dma_start(
                    wv.rearrange("p (i r) -> p i r", i=2),
                    wv8[2 * kp:2 * kp + 2, :, :].rearrange("i p r -> p i r"))
                wvt.append(wv)
                ws = dwp.tile([128, 2 * 512], FP8, tag=f"ws{kp}", name=f"ws{kp}")
                nc.sync.dma_start(
                    ws.rearrange("p (i r) -> p i r", i=2),
                    ws8[2 * kp:2 * kp + 2, :, :].rearrange("i p r -> p i r"))
                wst.append(ws)
            # rope tables (shared q/k; descale rides the evacuation copies)
            cos_sb = consts.tile([128, T], BF16, tag="cosb", name="cos_sb")
            sin_sb = consts.tile([128, T], BF16, tag="sinb", name="sin_sb")
            tables = ((cos_sb, cosb), (sin_sb, sinb))

            def load_tables(tsl):
                for dst, src in tables:
                    nc.sync.dma_start(dst[:, tsl], src[:, tsl])

            load_tables(slice(0, 512))
            xps_pre = [load_x(slice(512, 1024)), load_x(slice(1024, 1536))]
            load_tables(slice(512, 1024))
            load_tables(slice(1024, 1536))
            load_tables(slice(1536, 2048))
            mask_sb = consts.tile([128, 512], BF16, tag="mask", name="mask_sb")
            wcs_sb = consts.tile([128, 2 * C], BF16, tag="wcsb", name="wcs_sb")
            nc.sync.dma_start(mask_sb, mask[:, :])
            nc.sync.dma_start(wcs_sb, wcsb[:, :])

            for t in range(TC):
                tsl = slice(t * 512, (t + 1) * 512)
                if t in (1, 2):
                    xps = xps_pre[t - 1]
                elif t > 0:
                    xps = load_x(tsl)
                xv = [xp.rearrange("p (i t x) -> p i t x", i=2, t=2) for xp in xps[:2]]

                # 4 accumulating groups per side: T1 T2 T3a T3b; k side
                # reuses the q banks through the bufs=1 ring.
                def side(base, tags):
                    grp = [qkps.tile([128, 512], F32, tag=f"g{i}", name=f"g{i}")
                           for i in tags]
                    for g in range(4):
                        for kc in range(KC):
                            kp, i = kc // 2, kc % 2
                            dw3 = dwt[kc].rearrange("p (t n) -> p t n", t=2)
                            nc.tensor.matmul(
                                grp[g],
                                lhsT=dw3[:, :, (base + g) * 128:(base + g + 1) * 128],
                                rhs=xv[kp][:, i, :, :],
                                start=(kc == 0), stop=(kc == KC - 1),
                                perf_mode=DR)
                    return grp

                def vproj(tb):
                    # 64*v = (x8 + r8)' w8 + x8' s8  (fp8 DoubleRow passes)
                    blk = 4 * t + tb
                    vp = vps.tile([128, 256], F32, tag="vp", name="vp")
                    xrv = [xps[2].rearrange("p (i t x) -> p i t x", i=2, t=2),
                           xps[3].rearrange("p (i t x) -> p i t x", i=2, t=2)]
                    for pi, (xop, wop) in enumerate(
                            ((xv, wvt), (xrv, wvt), (xv, wst))):
                        for kc in range(KC):
                            kp, i = kc // 2, kc % 2
                            wv3 = wop[kp].rearrange(
                                "p (i t n) -> p i t n", i=2, t=2)
                            nc.tensor.matmul(
                                vp,
                                lhsT=xop[kp][:, i, :, tb * 128:(tb + 1) * 128],
                                rhs=wv3[:, i, :, :],
                                start=(pi == 0 and kc == 0),
                                stop=(pi == 2 and kc == KC - 1),
                                perf_mode=DR)
                    dst = va4[:, blk:HPG * KB:KB, 0:DH]
                    src = vp.rearrange("p (h d) -> p h d", h=HPG)
                    nc.scalar.copy(dst, src)

                def rope(grp, fin, desc, dve_t3b):
                    # evacuate with the descale; one k-side copy on DVE to
                    # balance the engines
                    sb = [psb.tile([128, 512], BF16, tag=f"qksb{i}", name=f"qksb{i}")
                          for i in range(4)]
                    for i in range(4):
                        if dve_t3b and i == 3:
                            nc.vector.tensor_scalar_mul(sb[i], grp[i], desc)
                        else:
                            nc.scalar.activation(
                                sb[i], grp[i], mybir.ActivationFunctionType.Copy,
                                scale=desc)
                    for ft in range(FT):
                        nc.vector.tensor_mul(
                            fin[ft][:, tsl], sb[ft], cos_sb[:, tsl])
                    for ft in range(FT):
                        t3 = sb[2 + ft]
                        eng = nc.gpsimd if ft == 0 else nc.vector
                        for po in (32, 96):
                            eng.tensor_mul(
                                t3[po:po + 32, :], t3[po:po + 32, :],
                                sin_sb[po:po + 32, tsl])
                            dst = fin[ft][po:po + 32, tsl]
                            nc.vector.tensor_add(dst, dst, t3[po:po + 32, :])

                qg = side(0, (0, 1, 2, 3))
                vproj(0)
                rope(qg, qfin, QDESC, False)
                vproj(1)
                kg = side(4, (4, 5, 0, 1))
                rope(kg, kfin, KDESC, True)
                vproj(2)
                vproj(3)

        # ============ attention + output projection =============
        with tc.tile_pool(name="sc_ps", bufs=2, space="PSUM") as scp, \
             tc.tile_pool(name="dg_ps", bufs=2, space="PSUM") as dgp, \
             tc.tile_pool(name="yt_ps", bufs=2, space="PSUM") as ytp, \
             tc.tile_pool(name="att_sb", bufs=4) as asb, \
             tc.tile_pool(name="dg_sb", bufs=4) as dsb, \
             tc.tile_pool(name="small_sb", bufs=4) as ssb, \
             tc.tile_pool(name="dram_scr", bufs=4, space="DRAM") as dsp, \
             tc.tile_pool(name="out_sb", bufs=3) as osbp:
            w3 = wcs_sb.rearrange("p (t n) -> p t n", t=2)

            def emit_unit(j, h):
                qsl = slice(j * 512, (j + 1) * 512)
                ft, off = h // 2, (h % 2) * 64
                hsl = slice(off, off + 64)
                vsl = lambda b: slice(h * VW + b * 65, h * VW + (b + 1) * 65)
                yp = ytp.tile([128, 512], F32, tag="yt", name="yt")
                # full off-diagonal key blocks, in pairs
                for pi in range(2 * j):
                    sp2 = scp.tile([128, 1024], F32, tag="sc", name="sc")
                    for half in range(2):
                        b = 2 * pi + half
                        nc.tensor.matmul(
                            sp2[:, half * 512:(half + 1) * 512],
                            lhsT=kfin[ft][hsl, b * 128:(b + 1) * 128],
                            rhs=qfin[ft][hsl, qsl],
                            start=True, stop=True)
                    pr2 = asb.tile([128, 1024], BF16, tag="pr", name="pr")
                    nc.scalar.activation(pr2, sp2, EXP)
                    for half in range(2):
                        b = 2 * pi + half
                        nc.tensor.matmul(
                            yp[0:DH + 1, :],
                            lhsT=vaug[:, vsl(b)],
                            rhs=pr2[:, half * 512:(half + 1) * 512],
                            start=(b == 0), stop=False,
                            skip_group_check=True)
                # diagonal band at 128-query granularity.
                # sub-chunk m needs key blocks b = 4j+beta, beta <= m;
                # d := m-beta.  DA: d=0 (masked); DB: d=1 + d=3; DC: d=2.
                DA = dgp.tile([128, 512], F32, tag="dg", name="dg")
                DB = dgp.tile([128, 512], F32, tag="dg", name="dg")
                DC = dgp.tile([128, 512], F32, tag="dg", name="dg")
                qb = j * 512

                def dmm(dst, b, m):
                    nc.tensor.matmul(
                        dst,
                        lhsT=kfin[ft][hsl, b * 128:(b + 1) * 128],
                        rhs=qfin[ft][hsl, qb + m * 128:qb + (m + 1) * 128],
                        start=True, stop=True)

                for m in range(4):
                    dmm(DA[:, m * 128:(m + 1) * 128], 4 * j + m, m)
                for m in range(1, 4):
                    dmm(DB[:, (m - 1) * 128:m * 128], 4 * j + m - 1, m)
                dmm(DB[:, 384:512], 4 * j, 3)
                for m in range(2, 4):
                    dmm(DC[:, (m - 2) * 128:(m - 1) * 128], 4 * j + m - 2, m)
                prA = dsb.tile([128, 512], BF16, tag="prd", name="prd")
                prB = dsb.tile([128, 512], BF16, tag="prd", name="prd")
                prC = dsb.tile([128, 512], BF16, tag="prd", name="prd")
                nc.scalar.activation(prA, DA, EXP)
                nc.scalar.activation(prB, DB, EXP)
                nc.scalar.activation(prC[:, 0:256], DC[:, 0:256], EXP)
                nc.vector.tensor_mul(prA, prA, mask_sb)
                for m in range(4):
                    for beta in range(m + 1):
                        d = m - beta
                        if d == 0:
                            rhs = prA[:, m * 128:(m + 1) * 128]
                        elif d == 1:
                            rhs = prB[:, (m - 1) * 128:m * 128]
                        elif d == 2:
                            rhs = prC[:, (m - 2) * 128:(m - 1) * 128]
                        else:
                            rhs = prB[:, 384:512]
                        nc.tensor.matmul(
                            yp[0:DH + 1, m * 128:(m + 1) * 128],
                            lhsT=vaug[:, vsl(4 * j + beta)],
                            rhs=rhs,
                            start=(j == 0 and beta == 0),
                            stop=(beta == m),
                            skip_group_check=True)
                # normalize: rec = 1/(8*sum pr), broadcast to 64 rows of
                # SBUF via a DRAM round trip (DMA queue has headroom)
                recs = ssb.tile([1, 512], F32, tag="recs", name="recs")
                nc.vector.reciprocal(recs, yp[DH:DH + 1, :])
                rec64 = ssb.tile([64, 512], F32, tag="rec64", name="rec64")
                nc.gpsimd.partition_broadcast(rec64, recs)
                if h % 2 == 0:
                    nc.vector.tensor_mul(
                        y3[0:64, h // 2, qsl], yp[0:DH, :], rec64)
                else:
                    yst = ssb.tile([64, 512], BF16, tag="yst", name="yst")
                    nc.vector.tensor_mul(yst, yp[0:DH, :], rec64)
                    nc.vector.tensor_copy(y3[64:128, h // 2, qsl], yst)

            def emit_cproj_piece(j, mp, tail):
                for s in range(2):
                    ob = osbp.tile([128, C], BF16, tag="ob", name="ob")
                    mi = 4 * j + 2 * mp + s
                    msl = slice(mi * 128, (mi + 1) * 128)
                    for n in range(2):
                        # at the tail the attention pools are idle: rotate
                        # through both PSUM rings for a deeper pipeline
                        if tail and (s + n) % 2 == 1:
                            op = scp.tile([128, 512], F32, tag="sc", name="op")
                        else:
                            op = dgp.tile([128, 512], F32, tag="dg", name="op")
                        for tt in range(2):
                            nc.tensor.matmul(
                                op,
                                lhsT=y3[:, tt, msl],
                                rhs=w3[:, tt, n * 512:(n + 1) * 512],
                                start=(tt == 0), stop=(tt == 1))
                        osl = ob[:, n * 512:(n + 1) * 512]
                        if tail and n == 0:
                            nc.scalar.copy(osl, op)
                        else:
                            nc.vector.tensor_copy(osl, op)
                    nc.sync.dma_start(out[msl, :], ob)

            # heavy chunks interleaved with thin ones (thin first, so
            # its short serial chain drains under the heavy unit's exp
            # stream); each pair's c_proj is spread through the NEXT
            # pair's units so its copies hide under exp work too
            pending = []
            for ja, jb in ((3, 0), (2, 1)):
                for h in range(HPG):
                    emit_unit(jb, h)
                    emit_unit(ja, h)
                    if pending:
                        emit_cproj_piece(*pending.pop(0), False)
                pending += [(ja, 0), (ja, 1), (jb, 0), (jb, 1)]
            for j, mp in pending:
                emit_cproj_piece(j, mp, True)


def _host_prep(x, Wq_down, Wk_down, Wv_down, Wq_up_c, Wq_up_e, Wk_up_c,
               Wk_up_e, Wv_up, Wc):
    """Build the per-core input maps."""
    bf = ml_dtypes.bfloat16
    f8 = ml_dtypes.float8_e4m3

    # rope cache, transposed: (32, T)
    inv_freq = 1.0 / (THETA ** (np.arange(0, DHE, 2, dtype=np.float64) / DHE))
    freqs = np.arange(T, dtype=np.float64)[:, None] * inv_freq[None, :]
    emb = np.concatenate((freqs, freqs), axis=-1)  # (T, 32)
    cosT = np.cos(emb).T  # (32, T)
    sinT = np.sin(emb).T

    # signed permutation: rot[2i] = -e[2i+1], rot[2i+1] = e[2i]
    P = np.zeros((DHE, DHE))
    for i in range(DHE // 2):
        P[2 * i, 2 * i + 1] = -1.0
        P[2 * i + 1, 2 * i] = 1.0

    ones32 = np.ones((32, T))
    zeros32 = np.zeros((32, T))
    # T1/T2 rows per head pair: [c(32) | e(32)] x2 -> cos rows [1,cos,1,cos]
    cos4 = np.concatenate([ones32, cosT, ones32, cosT], axis=0)
    # T3a/T3b rows: [0 | rot | 0 | rot] -> sin rows [0,sin,0,sin]
    sin4 = np.concatenate([zeros32, sinT, zeros32, sinT], axis=0)

    # mask for d=0 diagonal blocks: [128, 512] = 4 copies of lower triangle
    kk = np.arange(128)[:, None]
    qq = np.arange(128)[None, :]
    tri = (kk <= qq).astype(np.float64)
    mask_np = np.tile(tri, (1, 4))

    Wq_down = np.asarray(Wq_down, np.float64)
    Wk_down = np.asarray(Wk_down, np.float64)
    Wv_down = np.asarray(Wv_down, np.float64)
    Wq_up_c = np.asarray(Wq_up_c, np.float64)
    Wq_up_e = np.asarray(Wq_up_e, np.float64)
    Wk_up_c = np.asarray(Wk_up_c, np.float64)
    Wk_up_e = np.asarray(Wk_up_e, np.float64)
    Wv_up = np.asarray(Wv_up, np.float64)
    Wc = np.asarray(Wc, np.float64)
    Wq_rot = Wq_up_e @ P.T   # lat -> rot rows (before sin)
    Wk_rot = Wk_up_e @ P.T

    xTs, xRs = [], []
    for b in range(B):
        xT = np.ascontiguousarray(np.asarray(x[b], np.float64).T)
        x8 = xT.astype(f8)
        xTs.append(x8)
        xRs.append((xT - x8.astype(np.float64)).astype(f8))

    in_maps = []
    for core in range(8):
        b, hg = core // HG, core % HG
        # fused q/k down-proj weight [C, 1024]:
        # cols: qT1 qT2 qT3a qT3b kT1 kT2 kT3a kT3b (128 each);
        # T3a/T3b zero-pad the rot rows into partition alignment
        Weff = np.zeros((C, 1024))
        Wveff = np.zeros((C, 256))
        for hh in range(HPG):
            gh = hg * HPG + hh
            lsl = slice(gh * L, (gh + 1) * L)
            po = (hh % 2) * 64   # partition offset within tile
            ftq = (hh // 2) * 128
            rot_col = 256 + ftq + 32 + po  # T3a/T3b, rows 32-63 / 96-127
            Weff[:, ftq + po:ftq + po + 32] = Wq_down[:, lsl] @ Wq_up_c
            Weff[:, ftq + po + 32:ftq + po + 64] = Wq_down[:, lsl] @ Wq_up_e
            Weff[:, rot_col:rot_col + 32] = Wq_down[:, lsl] @ Wq_rot
            Weff[:, 512 + ftq + po:512 + ftq + po + 32] = Wk_down[:, lsl] @ Wk_up_c
            Weff[:, 512 + ftq + po + 32:512 + ftq + po + 64] = Wk_down[:, lsl] @ Wk_up_e
            Weff[:, 512 + rot_col:512 + rot_col + 32] = Wk_down[:, lsl] @ Wk_rot
            Wveff[:, hh * 64:(hh + 1) * 64] = Wv_down[:, lsl] @ Wv_up
        Weff *= WSC
        # DoubleRow pack: dw8[k, p, t*1024+n] = Weff[k*256 + t*128 + p, n]
        dw8 = Weff.reshape(KC, 2, 128, 1024).transpose(0, 2, 1, 3).reshape(
            KC, 128, 2 * 1024)
        Wv64 = Wveff * 64.0
        w8v = Wv64.astype(f8)
        s8v = Wv64 - w8v.astype(np.float64)
        wv8 = w8v.astype(np.float64).reshape(KC, 2, 128, 256).transpose(
            0, 2, 1, 3).reshape(KC, 128, 2 * 256)
        ws8 = s8v.reshape(KC, 2, 128, 256).transpose(
            0, 2, 1, 3).reshape(KC, 128, 2 * 256)
        # c_proj weights, packed over the 256 local y rows:
        # row (p, t) = head (2t + p//64), dim p%64
        wc_slice = Wc[hg * HPG * DH:(hg + 1) * HPG * DH, :]  # (256, C)
        wcsb = np.zeros((128, 2, C))
        for p64 in range(2):
            for t in range(2):
                hh = 2 * t + p64
                wcsb[p64 * 64:(p64 + 1) * 64, t, :] = \
                    wc_slice[hh * DH:(hh + 1) * DH, :]
        in_maps.append({
            "xT8": xTs[b],
            "xr8": xRs[b],
            "dw8": dw8.astype(f8),
            "wv8": wv8.astype(f8),
            "ws8": ws8.astype(f8),
            "cosb": cos4.astype(bf),
            "sinb": sin4.astype(bf),
            "mask": mask_np.astype(bf),
            "wcsb": wcsb.reshape(128, 2 * C).astype(bf),
        })
    return in_maps


LAST_RESULT = {}


def kernel(**inputs):
    inputs = {k: np.asarray(v, dtype=np.float32) for k, v in inputs.items()}
    nc = _build_nc()
    in_maps = _host_prep(**inputs)
    res = run_bass_kernel_spmd(nc, in_maps, core_ids=list(range(8)))
    LAST_RESULT.clear()
    LAST_RESULT.update(
        exec_time_ns=res.exec_time_ns,
        mean_exec_time_ns=res.mean_exec_time_ns,
        profile_json=res.profile_json,
    )
    parts = [r["out"].astype(np.float32) for r in res.results]
    out = np.stack([
        parts[0] + parts[1] + parts[2] + parts[3],
        parts[4] + parts[5] + parts[6] + parts[7],
    ])
    return out.astype(np.float32)


if __name__ == "__main__":
    rng = np.random.default_rng(0)
    ins = {
        "x": rng.standard_normal((B, T, C), dtype=np.float32),
        "Wq_down": rng.standard_normal((C, H * L), dtype=np.float32) * 0.02,
        "Wk_down": rng.standard_normal((C, H * L), dtype=np.float32) * 0.02,
        "Wv_down": rng.standard_normal((C, H * L), dtype=np.float32) * 0.02,
        "Wq_up_c": rng.standard_normal((L, DHE), dtype=np.float32) * 0.02,
        "Wq_up_e": rng.standard_normal((L, DHE), dtype=np.float32) * 0.02,
        "Wk_up_c": rng.standard_normal((L, DHE), dtype=np.float32) * 0.02,
        "Wk_up_e": rng.standard_normal((L, DHE), dtype=np.float32) * 0.02,
        "Wv_up": rng.standard_normal((L, DH), dtype=np.float32) * 0.02,
        "Wc": rng.standard_normal((C, C), dtype=np.float32) * 0.02,
    }
    y = kernel(**ins)
    print(y.shape, y.dtype, float(np.abs(y).mean()))
